# revision 1
# baseline (speedup 1.0000x reference)
"""CodeWiseAttention kernel for Trainium2 (8 NeuronCores, label-dim sharded).

m[b,n,:] = softmax(label_feature[n] @ x[b].T) @ x[b]

Sharding: label rows N=8922 split across 8 cores (1116/core, padded to 1152);
x replicated. Per core, per batch:
  mm1 (fp32r): S^T[l,n] = xT[e,l].T @ labelT[e,n]     (xT via PE transpose)
  exp on ScalarE: expS = exp(S - 30)                   (constant shift; cancels)
  mm2 (fp32r): Uaug^T[e',n] += xa[l,e'].T @ expS^T[l,n]  accumulated over l,
      where xa has a ones column so row 100 of Uaug = Z = sum_l expS.
  out: PE-transpose Uaug^T -> [n, e'], m = U / Z, DMA out.
"""
import numpy as np
from contextlib import ExitStack

import concourse.tile as tile
from concourse import bacc, mybir
from concourse.bass_utils import run_bass_kernel_spmd

F32 = mybir.dt.float32
F32R = mybir.dt.float32r

B, L, E = 8, 2500, 100
LP = 2520          # L padded with zero rows (zero rows add nothing to U or Z)
N_TOTAL = 8922
NCORES = 8
NS = 1116          # label rows per core (core 7: 1110 real)
NSP = 1152         # padded per-core label rows
LC = 126           # l-chunk rows (even: fp32r ISA needs even innermost counts)
NLC = LP // LC     # 20 l-chunks
NCH = 384          # n-chunk width (moving dim; >=256 keeps fp32r at full rate)
NJ = NSP // NCH    # 3 n-chunks
EA = E + 1         # x augmented with ones column
PSB = 512          # psum bank stride in f32 elements
EXP_BIAS = -30.0

TRACE = False
LAST_RESULT = None

_NC = []


def _build():
    nc = bacc.Bacc("TRN2", target_bir_lowering=False, debug=False)
    xa_d = nc.dram_tensor("xa", [B, LP, EA], F32R, kind="ExternalInput").ap()
    lab_d = nc.dram_tensor("lab", [NSP, E], F32R, kind="ExternalInput").ap()
    idr_d = nc.dram_tensor("idr", [128, 128], F32R, kind="ExternalInput").ap()
    idf_d = nc.dram_tensor("idf", [128, 128], F32, kind="ExternalInput").ap()
    m_d = nc.dram_tensor("m", [B, NSP, E], F32, kind="ExternalOutput").ap()

    with tile.TileContext(nc) as tc, ExitStack() as ctx:
        consts = ctx.enter_context(tc.tile_pool(name="consts", bufs=1))
        lab_pool = ctx.enter_context(tc.tile_pool(name="labp", bufs=2))
        xa_pool = ctx.enter_context(tc.tile_pool(name="xap", bufs=2))
        xt_pool = ctx.enter_context(tc.tile_pool(name="xtp", bufs=2))
        e_pool = ctx.enter_context(tc.tile_pool(name="ep", bufs=3))
        u_pool = ctx.enter_context(tc.tile_pool(name="up", bufs=3))
        o_pool = ctx.enter_context(tc.tile_pool(name="op", bufs=4))
        r_pool = ctx.enter_context(tc.tile_pool(name="rp", bufs=4))
        pstr = ctx.enter_context(tc.tile_pool(name="pstr", bufs=2, space="PSUM"))
        pss = ctx.enter_context(tc.tile_pool(name="pss", bufs=1, space="PSUM"))
        psm = ctx.enter_context(tc.tile_pool(name="psm", bufs=1, space="PSUM"))

        idr_sb = consts.tile([128, 128], F32R)
        nc.sync.dma_start(out=idr_sb[:], in_=idr_d)
        idf_sb = consts.tile([128, 128], F32)
        nc.sync.dma_start(out=idf_sb[:], in_=idf_d)
        bias_sb = consts.tile([128, 1], F32)
        nc.vector.memset(bias_sb[:], EXP_BIAS)

        # labelT [E, NSP] via PE transposes of 128-row label chunks
        labT = consts.tile([E, NSP], F32R)
        for k in range(NSP // 128):
            lsb = lab_pool.tile([128, E], F32R, tag="lab")
            nc.sync.dma_start(out=lsb[:], in_=lab_d[k * 128:(k + 1) * 128, :])
            tp = pstr.tile([128, 128], F32R, tag="tr")
            nc.tensor.transpose(tp[:E, :], lsb[:], idr_sb[:, :])
            nc.vector.tensor_copy(labT[:, k * 128:(k + 1) * 128], tp[:E, :])

        # prologue DMA for batch 0; per-batch DMA for b+1 is issued before
        # batch b's compute so the transfer hides under the c-loop
        xa_tiles = {}
        xa_tiles[0] = xa_pool.tile([LC, NLC, EA], F32R, tag="xa", name="xa_sb0")
        nc.sync.dma_start(
            out=xa_tiles[0][:], in_=xa_d[0].rearrange("(c p) e -> p c e", p=LC)
        )
        for b in range(B):
            xa_sb = xa_tiles.pop(b)
            if b + 1 < B:
                xa_tiles[b + 1] = xa_pool.tile(
                    [LC, NLC, EA], F32R, tag="xa", name=f"xa_sb{b+1}")
                nc.sync.dma_start(
                    out=xa_tiles[b + 1][:],
                    in_=xa_d[b + 1].rearrange("(c p) e -> p c e", p=LC),
                )
            # xT [E, LP] for this batch
            xT = xt_pool.tile([E, LP], F32R, tag="xt")
            for c in range(NLC):
                tp = pstr.tile([128, 128], F32R, tag="tr")
                nc.tensor.transpose(
                    tp[:E, :LC], xa_sb[:, c, 0:E], idr_sb[:LC, :LC]
                )
                nc.vector.tensor_copy(xT[:, c * LC:(c + 1) * LC], tp[:E, :LC])

            # two passes over l-chunks: j in {0,1}, then j=2. Halving the
            # S^T tile lets it double-buffer inside 8 PSUM banks, so
            # mm1(c+1) never waits on exp(c).
            u_sbs = []
            for jlo, jn in ((0, 2), (2, 1)):
                m_ps = psm.tile([EA, 2, PSB], F32, tag="m")
                for c in range(NLC):
                    s_ps = pss.tile([LC, 2, PSB], F32, tag="s")
                    for jj in range(jn):
                        nc.tensor.matmul(
                            s_ps[:, jj, 0:NCH],
                            xT[:, c * LC:(c + 1) * LC],
                            labT[:, (jlo + jj) * NCH:(jlo + jj + 1) * NCH],
                        )
                    e_sb = e_pool.tile([LC, 2, NCH], F32R, tag="e")
                    nc.scalar.activation(
                        e_sb[:, 0:jn, :], s_ps[:, 0:jn, 0:NCH],
                        mybir.ActivationFunctionType.Exp,
                        bias=bias_sb[:LC], scale=1.0,
                    )
                    for jj in range(jn):
                        nc.tensor.matmul(
                            m_ps[:, jj, 0:NCH],
                            xa_sb[:, c, :],
                            e_sb[:, jj, :],
                            start=(c == 0), stop=(c == NLC - 1),
                        )
                u_sb = u_pool.tile([EA, 2, NCH], F32, tag="u")
                nc.vector.tensor_copy(
                    u_sb[:, 0:jn, :], m_ps[:, 0:jn, 0:NCH]
                )
                u_sbs.append(u_sb)

            # out path: U^T -> transpose -> divide by Z -> DMA
            for k in range(NSP // 128):
                j, off = divmod(k * 128, NCH)
                u_src = u_sbs[0][:, j, off:off + 128] if j < 2 else \
                    u_sbs[1][:, 0, off:off + 128]
                tpo = pstr.tile([128, 128], F32, tag="tr")
                nc.tensor.transpose(
                    tpo[:, :EA], u_src, idf_sb[:EA, :EA]
                )
                rz = r_pool.tile([128, 1], F32, tag="r")
                nc.vector.reciprocal(rz[:], tpo[:, E:EA])
                o_sb = o_pool.tile([128, E], F32, tag="o")
                nc.vector.tensor_scalar_mul(o_sb[:], tpo[:, 0:E], rz[:])
                nc.sync.dma_start(
                    out=m_d[b, k * 128:(k + 1) * 128, :], in_=o_sb[:]
                )
    nc.compile()
    return nc


def _get_nc():
    if not _NC:
        _NC.append(_build())
    return _NC[0]


def kernel(x, label_feature):
    global LAST_RESULT
    x = np.ascontiguousarray(np.asarray(x, dtype=np.float32))
    lf = np.ascontiguousarray(np.asarray(label_feature, dtype=np.float32))
    assert x.shape == (B, L, E) and lf.shape == (N_TOTAL, E)

    xa = np.zeros((B, LP, EA), np.float32)
    xa[:, :L, :E] = x
    xa[:, :L, E] = 1.0
    ident = np.eye(128, dtype=np.float32)
    in_maps = []
    for r in range(NCORES):
        lo = r * NS
        hi = min(lo + NS, N_TOTAL)
        shard = np.zeros((NSP, E), np.float32)
        shard[: hi - lo] = lf[lo:hi]
        in_maps.append({"xa": xa, "lab": shard, "idr": ident, "idf": ident})

    nc = _get_nc()
    res = run_bass_kernel_spmd(
        nc, in_maps, core_ids=list(range(NCORES)), trace=TRACE
    )
    LAST_RESULT = res

    out = np.empty((B, N_TOTAL, E), np.float32)
    for r in range(NCORES):
        lo = r * NS
        hi = min(lo + NS, N_TOTAL)
        out[:, lo:hi, :] = res.results[r]["m"][:, : hi - lo, :]
    return out



# revision 2
# speedup vs baseline: 3.2658x; 3.2658x over previous
"""CodeWiseAttention kernel for Trainium2 (8 NeuronCores, label-dim sharded).

m[b,n,:] = softmax(label_feature[n] @ x[b].T) @ x[b]

Sharding: label rows N=8922 split across 8 cores (1116/core, padded to 1152);
x replicated. All transposes/padding done on host; on-chip per core, per batch:
  mm1 (fp16):  S[l,n] = xT[e,l].T @ labT[e,n]   per 128-row l-chunk, n in 3x384
  exp (ScalarE): e_sb = bf16(exp(S - 30))       one call per l-chunk (N=1152)
  mm2 (bf16):  m_ps[n,e'] += e_sb[l,n-chunk].T @ xa[l,e']  accumulated over
      l-chunks; xa has a ones column so col 100 of m_ps = Z = sum_l exp(S).
  out: m = U/Z on VectorE straight from [n,e] layout (no PE transposes), DMA.

fp16 for mm1 keeps score error ~4e-3 abs (exp amplifies score errors); bf16
for mm2 because exp values reach e^31 (fp16 would overflow). Measured rel
err vs f64 reference: ~5e-3 (threshold 2e-2).

Software pipeline over batches: iteration i runs mm1+exp of batch i
interleaved with mm2+output of batch i-1, so TensorE and ScalarE both stay
busy. PSUM: 2 x 3 banks mm1 scores (double-buffered) + 2 banks for the 9
m-accumulator slots (5 per bank, sequential start/stop groups) = 8 banks.
"""
import numpy as np
from contextlib import ExitStack

import concourse.tile as tile
from concourse import bacc, mybir
from concourse.bass_utils import run_bass_kernel_spmd

F32 = mybir.dt.float32
F16 = mybir.dt.float16
BF16 = mybir.dt.bfloat16

B, L, E = 8, 2500, 100
LP = 2560          # L padded to 20 chunks of 128 (zero rows contribute nothing)
NCHUNK = LP // 128
N_TOTAL = 8922
NCORES = 8
NS = 1116          # label rows per core (core 7: 1110 real)
NSP = 1152         # padded per-core label rows = 9*128 = 3*384
NCH = 384          # mm1 moving width
EA = E + 2         # x cols + ones col (100) + pad col (101)
PSB = 512          # psum bank stride in f32 elements
EXP_BIAS = -30.0

TRACE = False
LAST_RESULT = None

_NC = []


def _build():
    nc = bacc.Bacc("TRN2", target_bir_lowering=False, debug=False)
    xt_d = nc.dram_tensor("xt", [B, E, LP], F16, kind="ExternalInput").ap()
    xa_d = nc.dram_tensor("xa", [B, LP, EA], BF16, kind="ExternalInput").ap()
    lab_d = nc.dram_tensor("lab", [E, NSP], F16, kind="ExternalInput").ap()
    m_d = nc.dram_tensor("m", [B, NSP, E], F32, kind="ExternalOutput").ap()

    with tile.TileContext(nc) as tc, ExitStack() as ctx:
        consts = ctx.enter_context(tc.tile_pool(name="consts", bufs=1))
        xt_pool = ctx.enter_context(tc.tile_pool(name="xtp", bufs=2))
        xa_pool = ctx.enter_context(tc.tile_pool(name="xap", bufs=3))
        e_pool = ctx.enter_context(tc.tile_pool(name="ep", bufs=2))
        u_pool = ctx.enter_context(tc.tile_pool(name="up", bufs=2))
        o_pool = ctx.enter_context(tc.tile_pool(name="op", bufs=4))
        r_pool = ctx.enter_context(tc.tile_pool(name="rp", bufs=4))
        s_psum = ctx.enter_context(tc.tile_pool(name="sps", bufs=2, space="PSUM"))
        m_psum = ctx.enter_context(tc.tile_pool(name="mps", bufs=1, space="PSUM"))

        labT_sb = consts.tile([E, NSP], F16)
        nc.sync.dma_start(out=labT_sb[:], in_=lab_d)
        bias_sb = consts.tile([128, 1], F32)
        nc.vector.memset(bias_sb[:], EXP_BIAS)

        # 9 accumulator slots of 102 f32 cols packed 5-per-bank into 2 banks;
        # slots accumulate strictly sequentially so the whole-bank has_written
        # clear by a slot's first (start=True) matmul never hits a slot that
        # is still mid-accumulation.
        m_ps = m_psum.tile([128, 2, PSB], F32, name="m_ps")

        xt_tiles, xa_tiles, e_tiles = {}, {}, {}

        def dma_in(b):
            xt_tiles[b] = xt_pool.tile([E, LP], F16, tag="xt", name=f"xt{b}")
            nc.sync.dma_start(out=xt_tiles[b][:], in_=xt_d[b])
            xa_tiles[b] = xa_pool.tile(
                [128, NCHUNK, EA], BF16, tag="xa", name=f"xa{b}")
            nc.sync.dma_start(
                out=xa_tiles[b][:],
                in_=xa_d[b].rearrange("(c p) e -> p c e", p=128),
            )

        def mm2_slot(b, jn):
            e_prev = e_tiles[b]
            xa_sb = xa_tiles[b]
            q, r2 = divmod(jn, 5)
            j3, t = divmod(jn, 3)
            for c in range(NCHUNK):
                nc.tensor.matmul(
                    m_ps[:, q, r2 * EA:(r2 + 1) * EA],
                    e_prev[:, c, j3, t * 128:(t + 1) * 128],
                    xa_sb[:, c, :],
                    start=(c == 0), stop=(c == NCHUNK - 1),
                )

        dma_in(0)
        for it in range(B + 1):
            if it < B:
                if it + 1 < B:
                    dma_in(it + 1)
                e_tiles[it] = e_pool.tile(
                    [128, NCHUNK, 3, NCH], BF16, tag="e", name=f"e{it}")
                xt_sb = xt_tiles.pop(it)
                for k in range(NCHUNK):
                    s_ps = s_psum.tile([128, 3, PSB], F32, tag="s")
                    for j3 in range(3):
                        nc.tensor.matmul(
                            s_ps[:, j3, 0:NCH],
                            xt_sb[:, k * 128:(k + 1) * 128],
                            labT_sb[:, j3 * NCH:(j3 + 1) * NCH],
                        )
                    nc.scalar.activation(
                        e_tiles[it][:, k, :, :], s_ps[:, 0:3, 0:NCH],
                        mybir.ActivationFunctionType.Exp,
                        bias=bias_sb[:], scale=1.0,
                    )
                    if it > 0 and k % 2 == 1 and (k - 1) // 2 < 9:
                        mm2_slot(it - 1, (k - 1) // 2)
            else:
                for jn in range(9):
                    mm2_slot(it - 1, jn)

            if it > 0:
                b = it - 1
                u_sb = u_pool.tile([128, 2, PSB], F32, tag="u")
                nc.vector.tensor_copy(u_sb[:], m_ps[:])
                for jn in range(9):
                    q, r2 = divmod(jn, 5)
                    rz = r_pool.tile([128, 1], F32, tag="r")
                    nc.vector.reciprocal(
                        rz[:], u_sb[:, q, r2 * EA + E:r2 * EA + E + 1])
                    o_sb = o_pool.tile([128, E], F32, tag="o")
                    nc.vector.tensor_scalar_mul(
                        o_sb[:], u_sb[:, q, r2 * EA:r2 * EA + E], rz[:])
                    nc.sync.dma_start(
                        out=m_d[b, jn * 128:(jn + 1) * 128, :], in_=o_sb[:]
                    )
                del e_tiles[b], xa_tiles[b]
    nc.compile()
    return nc


def _get_nc():
    if not _NC:
        _NC.append(_build())
    return _NC[0]


def kernel(x, label_feature):
    global LAST_RESULT
    np_f16 = mybir.dt.np(F16)
    np_bf16 = mybir.dt.np(BF16)
    x = np.ascontiguousarray(np.asarray(x, dtype=np.float32))
    lf = np.ascontiguousarray(np.asarray(label_feature, dtype=np.float32))
    assert x.shape == (B, L, E) and lf.shape == (N_TOTAL, E)

    xt = np.zeros((B, E, LP), np_f16)
    xt[:, :, :L] = x.transpose(0, 2, 1).astype(np_f16)
    xa = np.zeros((B, LP, EA), np_bf16)
    xa[:, :L, :E] = x.astype(np_bf16)
    xa[:, :L, E] = 1.0

    in_maps = []
    for r in range(NCORES):
        lo = r * NS
        hi = min(lo + NS, N_TOTAL)
        shard = np.zeros((E, NSP), np_f16)
        shard[:, : hi - lo] = lf[lo:hi].T.astype(np_f16)
        in_maps.append({"xt": xt, "xa": xa, "lab": shard})

    nc = _get_nc()
    res = run_bass_kernel_spmd(
        nc, in_maps, core_ids=list(range(NCORES)), trace=TRACE
    )
    LAST_RESULT = res

    out = np.empty((B, N_TOTAL, E), np.float32)
    for r in range(NCORES):
        lo = r * NS
        hi = min(lo + NS, N_TOTAL)
        out[:, lo:hi, :] = res.results[r]["m"][:, : hi - lo, :]
    return out


# revision 5
# speedup vs baseline: 3.4067x; 1.0432x over previous
"""CodeWiseAttention kernel for Trainium2 (8 NeuronCores, label-dim sharded).

m[b,n,:] = softmax(label_feature[n] @ x[b].T) @ x[b]

Sharding: label rows N=8922 split across 8 cores (1116/core, padded to 1152);
x replicated. All transposes/padding done on host; on-chip per core, per batch,
fully streaming over 128-row l-chunks:
  mm1 (fp16):  S[l,n] = xT[e,l].T @ labT[e,n]   3 matmuls of n-width 384
  exp (ScalarE): e_sb = bf16(exp(S - 30))       one call per l-chunk (N=1152)
  mm2 (bf16):  m_ps[n,e'] += e_sb[l,n-chunk].T @ xa[l,e']  9 matmuls, l-chunk
      accumulated in PSUM; xa has a ones column so col 100 of m_ps = Z.
  out: m = U/Z on VectorE straight from [n,e] layout, DMA per 128-row slot.

fp16 for mm1 keeps score error ~4e-3 abs (exp amplifies score errors); bf16
for mm2 because exp values reach e^31 (fp16 would overflow). Measured rel
err vs f64 reference: ~4e-3 (threshold 2e-2).

The 9 m-accumulator slots (102 f32 each) pack 5-per-bank into 2 PSUM banks.
Only the first slot of each bank uses start=True (clearing the whole bank's
has_written bits); the other slots' first matmuls then overwrite-where-unset,
and all later chunks accumulate — this lets 9 interleaved accumulation groups
share 2 banks. ScalarE's exp stream (1 elem/cycle/lane, cost = free-dim
columns only) is the roofline: ~23.5us x 8 batches; mm1+mm2 fit underneath it
on the PE (weight loads fully hidden by the 64-deep reorder window).
PSUM: 2 x 3 banks scores (double-buffered) + 2 banks m-accumulators = 8.
"""
import numpy as np
from contextlib import ExitStack

import concourse.tile as tile
from concourse import bacc, mybir
from concourse.bass_utils import run_bass_kernel_spmd

F32 = mybir.dt.float32
F16 = mybir.dt.float16
BF16 = mybir.dt.bfloat16

B, L, E = 8, 2500, 100
LP = 2560          # L padded to 20 chunks of 128 (zero rows contribute nothing)
NCHUNK = LP // 128
N_TOTAL = 8922
NCORES = 8
NS = 1116          # label rows per core (core 7: 1110 real)
NSP = 1152         # padded per-core label rows = 9*128 = 3*384
NCH = 384          # mm1 moving width
EA = E + 2         # x cols + ones col (100) + pad col (101)
PSB = 512          # psum bank stride in f32 elements
EXP_BIAS = -30.0

TRACE = False
LAST_RESULT = None

_NC = []


def _build():
    nc = bacc.Bacc("TRN2", target_bir_lowering=False, debug=False)
    xt_d = nc.dram_tensor("xt", [B, E, LP], F16, kind="ExternalInput").ap()
    xa_d = nc.dram_tensor("xa", [B, LP, EA], BF16, kind="ExternalInput").ap()
    lab_d = nc.dram_tensor("lab", [E, NSP], F16, kind="ExternalInput").ap()
    m_d = nc.dram_tensor("m", [B, NSP, E], F32, kind="ExternalOutput").ap()

    with tile.TileContext(nc) as tc, ExitStack() as ctx:
        consts = ctx.enter_context(tc.tile_pool(name="consts", bufs=1))
        xt_pool = ctx.enter_context(tc.tile_pool(name="xtp", bufs=2))
        xa_pool = ctx.enter_context(tc.tile_pool(name="xap", bufs=2))
        e_pool = ctx.enter_context(tc.tile_pool(name="ep", bufs=4))
        u_pool = ctx.enter_context(tc.tile_pool(name="up", bufs=2))
        o_pool = ctx.enter_context(tc.tile_pool(name="op", bufs=4))
        r_pool = ctx.enter_context(tc.tile_pool(name="rp", bufs=4))
        s_psum = ctx.enter_context(tc.tile_pool(name="sps", bufs=2, space="PSUM"))
        m_psum = ctx.enter_context(tc.tile_pool(name="mps", bufs=1, space="PSUM"))

        labT_sb = consts.tile([E, NSP], F16)
        nc.sync.dma_start(out=labT_sb[:], in_=lab_d)
        bias_sb = consts.tile([128, 1], F32)
        nc.vector.memset(bias_sb[:], EXP_BIAS)

        m_ps = m_psum.tile([128, 2, PSB], F32, name="m_ps")
        # one-time init so the whole-tile U snapshot below never reads
        # uninitialized PSUM (the pack-gap columns are never matmul targets)
        nc.vector.memset(m_ps[:], 0.0)

        xt_tiles, xa_tiles = {}, {}

        def dma_in(b, split):
            xt_tiles[b] = xt_pool.tile([E, LP], F16, tag="xt", name=f"xt{b}")
            xa_tiles[b] = xa_pool.tile(
                [128, NCHUNK, EA], BF16, tag="xa", name=f"xa{b}")
            xa_r = xa_d[b].rearrange("(c p) e -> p c e", p=128)
            if split:
                # batch 0 is consumed immediately: fine-grained deps so the
                # first chunks' compute starts before the full DMA lands
                nc.sync.dma_start(
                    out=xt_tiles[b][:, 0:384], in_=xt_d[b][:, 0:384])
                nc.sync.dma_start(
                    out=xt_tiles[b][:, 384:LP], in_=xt_d[b][:, 384:LP])
                nc.sync.dma_start(
                    out=xa_tiles[b][:, 0:4, :], in_=xa_r[:, 0:4, :])
                nc.sync.dma_start(
                    out=xa_tiles[b][:, 4:NCHUNK, :], in_=xa_r[:, 4:NCHUNK, :])
            else:
                nc.sync.dma_start(out=xt_tiles[b][:], in_=xt_d[b])
                nc.sync.dma_start(out=xa_tiles[b][:], in_=xa_r)

        def mm2_chunk(b, c, e_sb):
            xa_sb = xa_tiles[b]
            for jn in range(9):
                q, r2 = divmod(jn, 5)
                j3, t = divmod(jn, 3)
                nc.tensor.matmul(
                    m_ps[:, q, r2 * EA:(r2 + 1) * EA],
                    e_sb[:, j3, t * 128:(t + 1) * 128],
                    xa_sb[:, c, :],
                    start=(c == 0 and r2 == 0), stop=(c == NCHUNK - 1),
                    skip_group_check=True,
                )

        dma_in(0, split=True)
        for b in range(B):
            if b + 1 < B:
                dma_in(b + 1, split=False)
            xt_sb = xt_tiles.pop(b)
            e_tiles = {}
            for c in range(NCHUNK):
                s_ps = s_psum.tile([128, 3, PSB], F32, tag="s")
                for j3 in range(3):
                    nc.tensor.matmul(
                        s_ps[:, j3, 0:NCH],
                        xt_sb[:, c * 128:(c + 1) * 128],
                        labT_sb[:, j3 * NCH:(j3 + 1) * NCH],
                    )
                e_tiles[c] = e_pool.tile(
                    [128, 3, NCH], BF16, tag="e", name=f"e{b}_{c}")
                nc.scalar.activation(
                    e_tiles[c][:], s_ps[:, 0:3, 0:NCH],
                    mybir.ActivationFunctionType.Exp,
                    bias=bias_sb[:], scale=1.0,
                )
                if c >= 1:
                    mm2_chunk(b, c - 1, e_tiles.pop(c - 1))
            mm2_chunk(b, NCHUNK - 1, e_tiles.pop(NCHUNK - 1))

            # out path: snapshot U to SBUF (also the WAR anchor that keeps
            # next batch's matmuls out of m_ps until it is read), then U/Z
            u_sb = u_pool.tile([128, 2, PSB], F32, tag="u")
            nc.vector.tensor_copy(u_sb[:], m_ps[:])
            for jn in range(9):
                q, r2 = divmod(jn, 5)
                rz = r_pool.tile([128, 1], F32, tag="r")
                nc.vector.reciprocal(
                    rz[:], u_sb[:, q, r2 * EA + E:r2 * EA + E + 1])
                o_sb = o_pool.tile([128, E], F32, tag="o")
                nc.vector.tensor_scalar_mul(
                    o_sb[:], u_sb[:, q, r2 * EA:r2 * EA + E], rz[:])
                nc.sync.dma_start(
                    out=m_d[b, jn * 128:(jn + 1) * 128, :], in_=o_sb[:]
                )
            del xa_tiles[b]
    nc.compile()
    return nc


def _get_nc():
    if not _NC:
        _NC.append(_build())
    return _NC[0]


def kernel(x, label_feature):
    global LAST_RESULT
    np_f16 = mybir.dt.np(F16)
    np_bf16 = mybir.dt.np(BF16)
    x = np.ascontiguousarray(np.asarray(x, dtype=np.float32))
    lf = np.ascontiguousarray(np.asarray(label_feature, dtype=np.float32))
    assert x.shape == (B, L, E) and lf.shape == (N_TOTAL, E)

    xt = np.zeros((B, E, LP), np_f16)
    xt[:, :, :L] = x.transpose(0, 2, 1).astype(np_f16)
    xa = np.zeros((B, LP, EA), np_bf16)
    xa[:, :L, :E] = x.astype(np_bf16)
    xa[:, :L, E] = 1.0

    in_maps = []
    for r in range(NCORES):
        lo = r * NS
        hi = min(lo + NS, N_TOTAL)
        shard = np.zeros((E, NSP), np_f16)
        shard[:, : hi - lo] = lf[lo:hi].T.astype(np_f16)
        in_maps.append({"xt": xt, "xa": xa, "lab": shard})

    nc = _get_nc()
    res = run_bass_kernel_spmd(
        nc, in_maps, core_ids=list(range(NCORES)), trace=TRACE
    )
    LAST_RESULT = res

    out = np.empty((B, N_TOTAL, E), np.float32)
    for r in range(NCORES):
        lo = r * NS
        hi = min(lo + NS, N_TOTAL)
        out[:, lo:hi, :] = res.results[r]["m"][:, : hi - lo, :]
    return out


# revision 6
# speedup vs baseline: 3.4324x; 1.0075x over previous
"""CodeWiseAttention kernel for Trainium2 (8 NeuronCores, label-dim sharded).

m[b,n,:] = softmax(label_feature[n] @ x[b].T) @ x[b]

Sharding: label rows N=8922 split across 8 cores (1116/core; core 7 pads 6
rows); x replicated. All transposes/padding done on host; on-chip per core,
per batch, fully streaming over 128-row l-chunks:
  mm1 (fp16):  S[l,n] = xT[e,l].T @ labT[e,n]   3 matmuls of n-width 372
  exp (ScalarE): e_sb = bf16(exp(S - 30))       one call per l-chunk (N=1116)
  mm2 (bf16):  m_ps[n,e'] += e_sb[l,n-chunk].T @ xa[l,e']  9 matmuls, l-chunk
      accumulated in PSUM; xa has a ones column so col 100 of m_ps = Z.
  out: m = U/Z on VectorE straight from [n,e] layout, DMA per 128-row slot.

fp16 for mm1 keeps score error ~4e-3 abs (exp amplifies score errors); bf16
for mm2 because exp values reach e^31 (fp16 would overflow). Measured rel
err vs f64 reference: ~4e-3 (threshold 2e-2).

The 9 m-accumulator slots (102 f32 each) pack 5-per-bank into 2 PSUM banks.
Only the first slot of each bank uses start=True (clearing the whole bank's
has_written bits); the other slots' first matmuls then overwrite-where-unset,
and all later chunks accumulate — 9 interleaved accumulation groups share 2
banks. ScalarE's exp stream (1 elem/cycle/lane, cost = free-dim columns) is
the roofline: ~22.7us x 8 batches; mm1+mm2 fit underneath it on the PE
(weight loads hidden by the PE's reorder window). The per-batch U snapshot to
SBUF doubles as the WAR anchor that keeps the next batch's matmuls out of
m_ps while VectorE reads it (PE-write + DVE-read of one PSUM bank is fatal).
PSUM: 2 x 3 banks scores (double-buffered) + 2 banks m-accumulators = 8.
"""
import numpy as np
from contextlib import ExitStack

import concourse.tile as tile
from concourse import bacc, mybir
from concourse.bass_utils import run_bass_kernel_spmd

F32 = mybir.dt.float32
F16 = mybir.dt.float16
BF16 = mybir.dt.bfloat16

B, L, E = 8, 2500, 100
LP = 2560          # L padded to 20 chunks of 128 (zero rows contribute nothing)
NCHUNK = LP // 128
N_TOTAL = 8922
NCORES = 8
NS = 1116          # label rows per core (core 7: 1110 real, 6 pad)
NCH = 372          # mm1 moving width (3 x 372 = 1116)
EA = E + 2         # x cols + ones col (100) + pad col (101)
PSB = 512          # psum bank stride in f32 elements
EXP_BIAS = -30.0
SLOT_P = [128] * 8 + [92]   # n-rows per m-accumulator slot (9 x 128 > 1116)

TRACE = False
LAST_RESULT = None

_NC = []


def _build():
    nc = bacc.Bacc("TRN2", target_bir_lowering=False, debug=False)
    xt_d = nc.dram_tensor("xt", [B, E, LP], F16, kind="ExternalInput").ap()
    xa_d = nc.dram_tensor("xa", [B, LP, EA], BF16, kind="ExternalInput").ap()
    lab_d = nc.dram_tensor("lab", [E, NS], F16, kind="ExternalInput").ap()
    m_d = nc.dram_tensor("m", [B, NS, E], F32, kind="ExternalOutput").ap()

    with tile.TileContext(nc) as tc, ExitStack() as ctx:
        consts = ctx.enter_context(tc.tile_pool(name="consts", bufs=1))
        xt_pool = ctx.enter_context(tc.tile_pool(name="xtp", bufs=2))
        xa_pool = ctx.enter_context(tc.tile_pool(name="xap", bufs=2))
        e_pool = ctx.enter_context(tc.tile_pool(name="ep", bufs=4))
        u_pool = ctx.enter_context(tc.tile_pool(name="up", bufs=2))
        o_pool = ctx.enter_context(tc.tile_pool(name="op", bufs=4))
        r_pool = ctx.enter_context(tc.tile_pool(name="rp", bufs=4))
        s_psum = ctx.enter_context(tc.tile_pool(name="sps", bufs=2, space="PSUM"))
        m_psum = ctx.enter_context(tc.tile_pool(name="mps", bufs=1, space="PSUM"))

        labT_sb = consts.tile([E, NS], F16)
        nc.sync.dma_start(out=labT_sb[:], in_=lab_d)
        bias_sb = consts.tile([128, 1], F32)
        nc.vector.memset(bias_sb[:], EXP_BIAS)

        m_ps = m_psum.tile([128, 2, PSB], F32, name="m_ps")
        # one-time init so the whole-tile U snapshot below never reads
        # uninitialized PSUM (the pack-gap columns are never matmul targets)
        nc.vector.memset(m_ps[:], 0.0)

        xt_tiles, xa_tiles = {}, {}

        def dma_in(b, pieces):
            xt_tiles[b] = xt_pool.tile([E, LP], F16, tag="xt", name=f"xt{b}")
            xa_tiles[b] = xa_pool.tile(
                [128, NCHUNK, EA], BF16, tag="xa", name=f"xa{b}")
            xa_r = xa_d[b].rearrange("(c p) e -> p c e", p=128)
            # chunk-granular pieces so compute can start before the full
            # transfer lands (matters for batch 0; 1 piece otherwise)
            xt_cuts = [0] + [c * 128 for c in pieces[0]] + [LP]
            for lo, hi in zip(xt_cuts[:-1], xt_cuts[1:]):
                nc.sync.dma_start(
                    out=xt_tiles[b][:, lo:hi], in_=xt_d[b][:, lo:hi])
            xa_cuts = [0] + list(pieces[1]) + [NCHUNK]
            for lo, hi in zip(xa_cuts[:-1], xa_cuts[1:]):
                nc.sync.dma_start(
                    out=xa_tiles[b][:, lo:hi, :], in_=xa_r[:, lo:hi, :])

        def mm2_chunk(b, c, e_sb):
            xa_sb = xa_tiles[b]
            e_flat = e_sb[:].rearrange("p a b -> p (a b)")
            for jn in range(9):
                q, r2 = divmod(jn, 5)
                np_ = SLOT_P[jn]
                nc.tensor.matmul(
                    m_ps[0:np_, q, r2 * EA:(r2 + 1) * EA],
                    e_flat[:, jn * 128:jn * 128 + np_],
                    xa_sb[:, c, :],
                    start=(c == 0 and r2 == 0), stop=(c == NCHUNK - 1),
                    skip_group_check=True,
                )

        def out_path(b, src):
            for jn in range(9):
                q, r2 = divmod(jn, 5)
                np_ = SLOT_P[jn]
                rz = r_pool.tile([128, 1], F32, tag="r")
                nc.vector.reciprocal(
                    rz[0:np_], src[0:np_, q, r2 * EA + E:r2 * EA + E + 1])
                o_sb = o_pool.tile([128, E], F32, tag="o")
                nc.vector.tensor_scalar_mul(
                    o_sb[0:np_], src[0:np_, q, r2 * EA:r2 * EA + E], rz[0:np_])
                half = np_ // 2
                nc.sync.dma_start(
                    out=m_d[b, jn * 128:jn * 128 + half, :],
                    in_=o_sb[0:half])
                nc.sync.dma_start(
                    out=m_d[b, jn * 128 + half:jn * 128 + np_, :],
                    in_=o_sb[half:np_])

        dma_in(0, ([2, 8], [2, 8]))
        for b in range(B):
            xt_sb = xt_tiles.pop(b)
            e_tiles = {}
            for c in range(NCHUNK):
                if c == 10 and b + 1 < B:
                    dma_in(b + 1, ([], []))
                s_ps = s_psum.tile([128, 3, PSB], F32, tag="s")
                for j3 in range(3):
                    nc.tensor.matmul(
                        s_ps[:, j3, 0:NCH],
                        xt_sb[:, c * 128:(c + 1) * 128],
                        labT_sb[:, j3 * NCH:(j3 + 1) * NCH],
                    )
                e_tiles[c] = e_pool.tile(
                    [128, 3, NCH], BF16, tag="e", name=f"e{b}_{c}")
                nc.scalar.activation(
                    e_tiles[c][:], s_ps[:, 0:3, 0:NCH],
                    mybir.ActivationFunctionType.Exp,
                    bias=bias_sb[:], scale=1.0,
                )
                if c >= 1:
                    mm2_chunk(b, c - 1, e_tiles.pop(c - 1))
            mm2_chunk(b, NCHUNK - 1, e_tiles.pop(NCHUNK - 1))

            if b + 1 < B:
                # snapshot U to SBUF: also the WAR anchor that keeps the next
                # batch's matmuls out of m_ps until VectorE has read it
                u_sb = u_pool.tile([128, 2, PSB], F32, tag="u")
                nc.vector.tensor_copy(u_sb[:], m_ps[:])
                out_path(b, u_sb)
            else:
                # last batch: no later matmuls, safe to read PSUM directly
                out_path(b, m_ps)
            del xa_tiles[b]
    nc.compile()
    return nc


def _get_nc():
    if not _NC:
        _NC.append(_build())
    return _NC[0]


def kernel(x, label_feature):
    global LAST_RESULT
    np_f16 = mybir.dt.np(F16)
    np_bf16 = mybir.dt.np(BF16)
    x = np.ascontiguousarray(np.asarray(x, dtype=np.float32))
    lf = np.ascontiguousarray(np.asarray(label_feature, dtype=np.float32))
    assert x.shape == (B, L, E) and lf.shape == (N_TOTAL, E)

    xt = np.zeros((B, E, LP), np_f16)
    xt[:, :, :L] = x.transpose(0, 2, 1).astype(np_f16)
    xa = np.zeros((B, LP, EA), np_bf16)
    xa[:, :L, :E] = x.astype(np_bf16)
    xa[:, :L, E] = 1.0

    in_maps = []
    for r in range(NCORES):
        lo = r * NS
        hi = min(lo + NS, N_TOTAL)
        shard = np.zeros((E, NS), np_f16)
        shard[:, : hi - lo] = lf[lo:hi].T.astype(np_f16)
        in_maps.append({"xt": xt, "xa": xa, "lab": shard})

    nc = _get_nc()
    res = run_bass_kernel_spmd(
        nc, in_maps, core_ids=list(range(NCORES)), trace=TRACE
    )
    LAST_RESULT = res

    out = np.empty((B, N_TOTAL, E), np.float32)
    for r in range(NCORES):
        lo = r * NS
        hi = min(lo + NS, N_TOTAL)
        out[:, lo:hi, :] = res.results[r]["m"][:, : hi - lo, :]
    return out


# revision 9
# speedup vs baseline: 3.4828x; 1.0147x over previous
"""CodeWiseAttention kernel for Trainium2 (8 NeuronCores, label-dim sharded).

m[b,n,:] = softmax(label_feature[n] @ x[b].T) @ x[b]

Sharding: label rows N=8922 split across 8 cores (1116/core; core 7 pads 6
rows); x replicated. All transposes/padding done on host; on-chip per core,
per batch, fully streaming over 128-row l-chunks:
  mm1 (fp16):  S[l,n] = xT[e,l].T @ labT[e,n]   3 matmuls of n-width 372
  exp (ScalarE): e_sb = bf16(exp(S - 30))       one call per l-chunk (N=1116)
  mm2 (bf16):  m_ps[n,e'] += e_sb[l,n-chunk].T @ xa[l,e']  9 matmuls, l-chunk
      accumulated in PSUM; xa has a ones column so col 100 of m_ps = Z.
  out: m = U/Z on VectorE straight from [n,e] layout, DMA per 128-row slot.

fp16 for mm1 keeps score error ~4e-3 abs (exp amplifies score errors); bf16
for mm2 because exp values reach e^31 (fp16 would overflow). Measured rel
err vs f64 reference: ~4e-3 (threshold 2e-2).

The 9 m-accumulator slots (102 f32 each) pack 5-per-bank into 2 PSUM banks.
Only the first slot of each bank uses start=True (clearing the whole bank's
has_written bits); the other slots' first matmuls then overwrite-where-unset,
and all later chunks accumulate — 9 interleaved accumulation groups share 2
banks. ScalarE's exp stream (1 elem/cycle/lane, cost = free-dim columns) is
the roofline: ~22.7us x 8 batches; mm1+mm2 fit underneath it on the PE
(weight loads hidden by the PE's reorder window). The per-batch U snapshot to
SBUF doubles as the WAR anchor that keeps the next batch's matmuls out of
m_ps while VectorE reads it (PE-write + DVE-read of one PSUM bank is fatal).
PSUM: 2 x 3 banks scores (double-buffered) + 2 banks m-accumulators = 8.
"""
import numpy as np
from contextlib import ExitStack

import concourse.tile as tile
from concourse import bacc, mybir
from concourse.bass_utils import run_bass_kernel_spmd

F32 = mybir.dt.float32
F16 = mybir.dt.float16
BF16 = mybir.dt.bfloat16

B, L, E = 8, 2500, 100
LP = 2560          # L padded to 20 chunks of 128 (zero rows contribute nothing)
NCHUNK = LP // 128
N_TOTAL = 8922
NCORES = 8
NS = 1116          # label rows per core (core 7: 1110 real, 6 pad)
NCH = 372          # mm1 moving width (3 x 372 = 1116)
EA = E + 2         # x cols + ones col (100) + pad col (101)
PSB = 512          # psum bank stride in f32 elements
EXP_BIAS = -30.0
SLOT_P = [128] * 8 + [92]   # n-rows per m-accumulator slot (9 x 128 > 1116)

TRACE = False
LAST_RESULT = None

_NC = []


def _build():
    nc = bacc.Bacc("TRN2", target_bir_lowering=False, debug=False)
    xt_d = nc.dram_tensor("xt", [B, E, LP], F16, kind="ExternalInput").ap()
    xa_d = nc.dram_tensor("xa", [B, LP, EA], BF16, kind="ExternalInput").ap()
    lab_d = nc.dram_tensor("lab", [E, NS], F16, kind="ExternalInput").ap()
    m_d = nc.dram_tensor("m", [B, NS, E], F32, kind="ExternalOutput").ap()

    with tile.TileContext(nc) as tc, ExitStack() as ctx:
        consts = ctx.enter_context(tc.tile_pool(name="consts", bufs=1))
        xt_pool = ctx.enter_context(tc.tile_pool(name="xtp", bufs=2))
        xa_pool = ctx.enter_context(tc.tile_pool(name="xap", bufs=2))
        e_pool = ctx.enter_context(tc.tile_pool(name="ep", bufs=4))
        u_pool = ctx.enter_context(tc.tile_pool(name="up", bufs=2))
        o_pool = ctx.enter_context(tc.tile_pool(name="op", bufs=4))
        r_pool = ctx.enter_context(tc.tile_pool(name="rp", bufs=4))
        s_psum = ctx.enter_context(tc.tile_pool(name="sps", bufs=2, space="PSUM"))
        m_psum = ctx.enter_context(tc.tile_pool(name="mps", bufs=1, space="PSUM"))

        labT_sb = consts.tile([E, NS], F16)
        for j3 in range(3):
            nc.sync.dma_start(
                out=labT_sb[:, j3 * NCH:(j3 + 1) * NCH],
                in_=lab_d[:, j3 * NCH:(j3 + 1) * NCH])
        bias_sb = consts.tile([128, 1], F32)
        nc.vector.memset(bias_sb[:], EXP_BIAS)

        m_ps = m_psum.tile([128, 2, PSB], F32, name="m_ps")
        # one-time init so the whole-tile U snapshot below never reads
        # uninitialized PSUM (the pack-gap columns are never matmul targets)
        nc.vector.memset(m_ps[:], 0.0)

        xt_tiles, xa_tiles = {}, {}

        def dma_in(b, pieces):
            xt_tiles[b] = xt_pool.tile([E, LP], F16, tag="xt", name=f"xt{b}")
            xa_tiles[b] = xa_pool.tile(
                [128, NCHUNK, EA], BF16, tag="xa", name=f"xa{b}")
            xa_r = xa_d[b].rearrange("(c p) e -> p c e", p=128)
            # chunk-granular pieces so compute can start before the full
            # transfer lands (matters for batch 0; 1 piece otherwise)
            xt_cuts = [0] + [c * 128 for c in pieces[0]] + [LP]
            for lo, hi in zip(xt_cuts[:-1], xt_cuts[1:]):
                nc.sync.dma_start(
                    out=xt_tiles[b][:, lo:hi], in_=xt_d[b][:, lo:hi])
            xa_cuts = [0] + list(pieces[1]) + [NCHUNK]
            for lo, hi in zip(xa_cuts[:-1], xa_cuts[1:]):
                nc.sync.dma_start(
                    out=xa_tiles[b][:, lo:hi, :], in_=xa_r[:, lo:hi, :])

        def mm2_chunk(b, c, e_sb):
            xa_sb = xa_tiles[b]
            e_flat = e_sb[:].rearrange("p a b -> p (a b)")
            for jn in range(9):
                q, r2 = divmod(jn, 5)
                np_ = SLOT_P[jn]
                nc.tensor.matmul(
                    m_ps[0:np_, q, r2 * EA:(r2 + 1) * EA],
                    e_flat[:, jn * 128:jn * 128 + np_],
                    xa_sb[:, c, :],
                    start=(c == 0 and r2 == 0), stop=(c == NCHUNK - 1),
                    skip_group_check=True,
                )

        def out_path(b, src):
            # one strided reciprocal over all 9 Z columns (slot (1,4) is the
            # memset-zero dummy -> inf, never read), 9 scaled copies into one
            # staging tile, and only two DMA triggers (each ~0.6us of Sync
            # sequencer time, which would otherwise serialize the tail)
            rz = r_pool.tile([128, 2, 5], F32, tag="r")
            nc.vector.reciprocal(rz[:], src[:, :, E:EA * 5:EA])
            o_all = o_pool.tile([128, 9, E], F32, tag="o")
            for jn in range(9):
                q, r2 = divmod(jn, 5)
                np_ = SLOT_P[jn]
                nc.vector.tensor_scalar_mul(
                    o_all[0:np_, jn, :], src[0:np_, q, r2 * EA:r2 * EA + E],
                    rz[0:np_, q, r2:r2 + 1])
            nc.sync.dma_start(
                out=m_d[b, 0:1024, :].rearrange("(s p) e -> p s e", p=128),
                in_=o_all[:, 0:8, :])
            nc.sync.dma_start(
                out=m_d[b, 1024:NS, :], in_=o_all[0:NS - 1024, 8, :])

        dma_in(0, ([2, 4, 8], [2, 4, 8]))
        for b in range(B):
            xt_sb = xt_tiles.pop(b)
            e_tiles = {}
            for c in range(NCHUNK):
                if c == 10 and b + 1 < B:
                    dma_in(b + 1, ([], []))
                s_ps = s_psum.tile([128, 3, PSB], F32, tag="s")
                for j3 in range(3):
                    nc.tensor.matmul(
                        s_ps[:, j3, 0:NCH],
                        xt_sb[:, c * 128:(c + 1) * 128],
                        labT_sb[:, j3 * NCH:(j3 + 1) * NCH],
                    )
                e_tiles[c] = e_pool.tile(
                    [128, 3, NCH], BF16, tag="e", name=f"e{b}_{c}")
                nc.scalar.activation(
                    e_tiles[c][:], s_ps[:, 0:3, 0:NCH],
                    mybir.ActivationFunctionType.Exp,
                    bias=bias_sb[:], scale=1.0,
                )
                if c >= 1:
                    mm2_chunk(b, c - 1, e_tiles.pop(c - 1))
            mm2_chunk(b, NCHUNK - 1, e_tiles.pop(NCHUNK - 1))

            if b + 1 < B:
                # snapshot U to SBUF: also the WAR anchor that keeps the next
                # batch's matmuls out of m_ps until VectorE has read it
                u_sb = u_pool.tile([128, 2, PSB], F32, tag="u")
                nc.vector.tensor_copy(u_sb[:], m_ps[:])
                out_path(b, u_sb)
            else:
                # last batch: no later matmuls, safe to read PSUM directly
                out_path(b, m_ps)
            del xa_tiles[b]
    nc.compile()
    return nc


def _get_nc():
    if not _NC:
        _NC.append(_build())
    return _NC[0]


def kernel(x, label_feature):
    global LAST_RESULT
    np_f16 = mybir.dt.np(F16)
    np_bf16 = mybir.dt.np(BF16)
    x = np.ascontiguousarray(np.asarray(x, dtype=np.float32))
    lf = np.ascontiguousarray(np.asarray(label_feature, dtype=np.float32))
    assert x.shape == (B, L, E) and lf.shape == (N_TOTAL, E)

    xt = np.zeros((B, E, LP), np_f16)
    xt[:, :, :L] = x.transpose(0, 2, 1).astype(np_f16)
    xa = np.zeros((B, LP, EA), np_bf16)
    xa[:, :L, :E] = x.astype(np_bf16)
    xa[:, :L, E] = 1.0

    in_maps = []
    for r in range(NCORES):
        lo = r * NS
        hi = min(lo + NS, N_TOTAL)
        shard = np.zeros((E, NS), np_f16)
        shard[:, : hi - lo] = lf[lo:hi].T.astype(np_f16)
        in_maps.append({"xt": xt, "xa": xa, "lab": shard})

    nc = _get_nc()
    res = run_bass_kernel_spmd(
        nc, in_maps, core_ids=list(range(NCORES)), trace=TRACE
    )
    LAST_RESULT = res

    out = np.empty((B, N_TOTAL, E), np.float32)
    for r in range(NCORES):
        lo = r * NS
        hi = min(lo + NS, N_TOTAL)
        out[:, lo:hi, :] = res.results[r]["m"][:, : hi - lo, :]
    return out


# revision 14
# speedup vs baseline: 3.7837x; 1.0864x over previous
"""CodeWiseAttention kernel for Trainium2 (8 NeuronCores, label-dim sharded).

m[b,n,:] = softmax(label_feature[n] @ x[b].T) @ x[b]

Sharding: label rows N=8922 split across 8 cores (1116/core; core 7 pads 6
rows); x replicated. All transposes/padding done on host; on-chip per core,
one global stream of 160 l-chunks (8 batches x 20 chunks of 128 rows):
  mm1 (fp16):  S[l,n] = xT[e,l].T @ labT[e,n]   3 matmuls of n-width 372
  exp (ScalarE): e_sb = bf16(exp(S - 30))       one call per l-chunk (N=1116)
  mm2 (bf16):  m_ps[n,e'] += e_sb[l,n-chunk].T @ xa[l,e']  9 matmuls, lagging
      the exp stream by 2 chunks; xa has a ones column so col 100 of U = Z.
  out: m = U/Z on VectorE straight from [n,e] layout, 2 DMA triggers/batch.

ScalarE's exp stream (1 elem/cycle/lane, cost = free-dim columns) is the
roofline: ~152us of pure streaming + per-call overhead. Everything else is
shaped to keep ScalarE gapless: mm1 of chunk g is emitted BEFORE mm2 of
chunk g-2 so the PE (in-order queue) produces scores the moment the s-buffer
frees; weight loads hide under matmuls via the PE's reorder window; input
DMAs trigger from the idle GpSimd queue and outputs from Sync (each trigger
costs ~0.6us of sequencer time); VectorE does the U/Z division.

fp16 for mm1 keeps score error ~4e-3 abs (exp amplifies score errors); bf16
for mm2 because exp values reach e^31 (fp16 would overflow). Measured rel
err vs f64 reference: ~4e-3 (threshold 2e-2).

The 9 m-accumulator slots (102 f32 cols) pack 5-per-bank into 2 PSUM banks.
Only the first slot of each bank uses start=True (clearing the whole bank's
has_written bits); the other slots' first matmuls then overwrite-where-unset
and all later chunks accumulate — 9 interleaved accumulation groups share 2
banks. The per-batch U snapshot to SBUF doubles as the WAR anchor that keeps
the next batch's matmuls out of m_ps while VectorE reads it (PE-write +
DVE-read of one PSUM bank is fatal; reading uninitialized PSUM is fatal too,
hence the one-time memset). PSUM: 2 x 3 banks scores + 2 banks U = 8.
"""
import numpy as np
from contextlib import ExitStack

import concourse.tile as tile
from concourse import bacc, mybir
from concourse.bass_utils import run_bass_kernel_spmd

F32 = mybir.dt.float32
F16 = mybir.dt.float16
BF16 = mybir.dt.bfloat16

B, L, E = 8, 2500, 100
LP = 2560          # L padded to 20 chunks of 128 (zero rows contribute nothing)
NCHUNK = LP // 128
NG = B * NCHUNK
N_TOTAL = 8922
NCORES = 8
NS = 1116          # label rows per core (core 7: 1110 real, 6 pad)
NCH = 372          # mm1 moving width (3 x 372 = 1116)
EA = E + 2         # x cols + ones col (100) + pad col (101)
PSB = 512          # psum bank stride in f32 elements
EXP_BIAS = -30.0
SLOT_P = [128] * 8 + [92]   # n-rows per m-accumulator slot (9 x 128 > 1116)

TRACE = False
LAST_RESULT = None

_NC = []


def _build():
    nc = bacc.Bacc("TRN2", target_bir_lowering=False, debug=False)
    xt_d = nc.dram_tensor("xt", [B, E, LP], F16, kind="ExternalInput").ap()
    xa_d = nc.dram_tensor("xa", [B, LP, EA], BF16, kind="ExternalInput").ap()
    lab_d = nc.dram_tensor("lab", [E, NS], F16, kind="ExternalInput").ap()
    m_d = nc.dram_tensor("m", [B, NS, E], F32, kind="ExternalOutput").ap()

    with tile.TileContext(nc) as tc, ExitStack() as ctx:
        consts = ctx.enter_context(tc.tile_pool(name="consts", bufs=1))
        xt_pool = ctx.enter_context(tc.tile_pool(name="xtp", bufs=2))
        xa_pool = ctx.enter_context(tc.tile_pool(name="xap", bufs=2))
        e_pool = ctx.enter_context(tc.tile_pool(name="ep", bufs=4))
        u_pool = ctx.enter_context(tc.tile_pool(name="up", bufs=2))
        o_pool = ctx.enter_context(tc.tile_pool(name="op", bufs=2))
        r_pool = ctx.enter_context(tc.tile_pool(name="rp", bufs=2))
        s_psum = ctx.enter_context(tc.tile_pool(name="sps", bufs=2, space="PSUM"))
        m_psum = ctx.enter_context(tc.tile_pool(name="mps", bufs=1, space="PSUM"))

        bias_sb = consts.tile([128, 1], F32)
        nc.vector.memset(bias_sb[:], EXP_BIAS)
        labT_sb = consts.tile([E, NS], F16)

        m_ps = m_psum.tile([128, 2, PSB], F32, name="m_ps")
        # one-time init so whole-tile reads below never touch uninitialized
        # PSUM; 1.0 (not 0) so the dummy-slot 1/Z stays finite
        nc.vector.memset(m_ps[:], 1.0)

        xt_tiles, xa_tiles, e_tiles = {}, {}, {}

        def dma_in(b):
            xt_tiles[b] = xt_pool.tile([E, LP], F16, tag="xt", name=f"xt{b}")
            xa_tiles[b] = xa_pool.tile(
                [128, NCHUNK, EA], BF16, tag="xa", name=f"xa{b}")
            nc.gpsimd.dma_start(out=xt_tiles[b][:], in_=xt_d[b])
            nc.gpsimd.dma_start(
                out=xa_tiles[b][:],
                in_=xa_d[b].rearrange("(c p) e -> p c e", p=128))

        def mm2_chunk(b, c, e_sb):
            xa_sb = xa_tiles[b]
            e_flat = e_sb[:].rearrange("p a b -> p (a b)")
            for jn in range(9):
                q, r2 = divmod(jn, 5)
                np_ = SLOT_P[jn]
                nc.tensor.matmul(
                    m_ps[0:np_, q, r2 * EA:(r2 + 1) * EA],
                    e_flat[:, jn * 128:jn * 128 + np_],
                    xa_sb[:, c, :],
                    start=(c == 0 and r2 == 0), stop=(c == NCHUNK - 1),
                    skip_group_check=True,
                )

        def out_path(b, src):
            # one strided reciprocal over all 10 Z columns (slot (1,4) is a
            # dummy), one broadcast multiply for U/Z, two DMA triggers
            rz = r_pool.tile([128, 2, 5], F32, tag="r")
            nc.vector.reciprocal(rz[:], src[:, :, E:EA * 5:EA])
            u4 = src[:, :, 0:EA * 5].rearrange(
                "p q (r c) -> p q r c", r=5)[:, :, :, 0:E]
            o_flat = o_pool.tile([128, 10 * E], F32, tag="o")
            o4 = o_flat[:].rearrange("p (q r c) -> p q r c", q=2, r=5)
            nc.vector.tensor_tensor(
                o4, u4, rz[:].to_broadcast([128, 2, 5, E]),
                mybir.AluOpType.mult)
            o3 = o_flat[:].rearrange("p (s c) -> p s c", s=10)
            nc.sync.dma_start(
                out=m_d[b, 0:1024, :].rearrange("(s p) e -> p s e", p=128),
                in_=o3[:, 0:8, :])
            nc.sync.dma_start(
                out=m_d[b, 1024:NS, :], in_=o3[0:NS - 1024, 8, :])

        def finish_batch(b):
            if b + 1 < B:
                # U snapshot: the WAR anchor keeping batch b+1's matmuls out
                # of m_ps until VectorE has read batch b's result
                u_sb = u_pool.tile([128, 2, PSB], F32, tag="u")
                nc.vector.tensor_copy(u_sb[:], m_ps[:])
                out_path(b, u_sb)
            else:
                out_path(b, m_ps)  # last batch: no later matmuls
            del xa_tiles[b]

        # prologue triggers ordered so chunk 0's deps (xt piece 1, labT)
        # land first
        xt_tiles[0] = xt_pool.tile([E, LP], F16, tag="xt", name="xt0")
        xa_tiles[0] = xa_pool.tile(
            [128, NCHUNK, EA], BF16, tag="xa", name="xa0")
        xa0_r = xa_d[0].rearrange("(c p) e -> p c e", p=128)
        nc.gpsimd.dma_start(out=xt_tiles[0][:, 0:256], in_=xt_d[0][:, 0:256])
        nc.gpsimd.dma_start(out=labT_sb[:], in_=lab_d)
        nc.gpsimd.dma_start(out=xa_tiles[0][:, 0:2, :], in_=xa0_r[:, 0:2, :])
        nc.gpsimd.dma_start(out=xt_tiles[0][:, 256:LP], in_=xt_d[0][:, 256:LP])
        nc.gpsimd.dma_start(
            out=xa_tiles[0][:, 2:NCHUNK, :], in_=xa0_r[:, 2:NCHUNK, :])
        for g in range(NG + 2):
            b, c = divmod(g, NCHUNK)
            if g < NG:
                if c == 10 and b + 1 < B:
                    dma_in(b + 1)
                xt_sb = xt_tiles[b]
                s_ps = s_psum.tile([128, 3, PSB], F32, tag="s")
                for j3 in range(3):
                    nc.tensor.matmul(
                        s_ps[:, j3, 0:NCH],
                        xt_sb[:, c * 128:(c + 1) * 128],
                        labT_sb[:, j3 * NCH:(j3 + 1) * NCH],
                    )
                e_tiles[g] = e_pool.tile(
                    [128, 3, NCH], BF16, tag="e", name=f"e{g}")
                nc.scalar.activation(
                    e_tiles[g][:], s_ps[:, 0:3, 0:NCH],
                    mybir.ActivationFunctionType.Exp,
                    bias=bias_sb[:], scale=1.0,
                )
                if c == NCHUNK - 1:
                    del xt_tiles[b]
            if g >= 2:
                b2, c2 = divmod(g - 2, NCHUNK)
                mm2_chunk(b2, c2, e_tiles.pop(g - 2))
                if c2 == NCHUNK - 1:
                    finish_batch(b2)
    nc.compile()
    return nc


def _get_nc():
    if not _NC:
        _NC.append(_build())
    return _NC[0]


def kernel(x, label_feature):
    global LAST_RESULT
    np_f16 = mybir.dt.np(F16)
    np_bf16 = mybir.dt.np(BF16)
    x = np.ascontiguousarray(np.asarray(x, dtype=np.float32))
    lf = np.ascontiguousarray(np.asarray(label_feature, dtype=np.float32))
    assert x.shape == (B, L, E) and lf.shape == (N_TOTAL, E)

    xt = np.zeros((B, E, LP), np_f16)
    xt[:, :, :L] = x.transpose(0, 2, 1).astype(np_f16)
    xa = np.zeros((B, LP, EA), np_bf16)
    xa[:, :L, :E] = x.astype(np_bf16)
    xa[:, :L, E] = 1.0

    in_maps = []
    for r in range(NCORES):
        lo = r * NS
        hi = min(lo + NS, N_TOTAL)
        shard = np.zeros((E, NS), np_f16)
        shard[:, : hi - lo] = lf[lo:hi].T.astype(np_f16)
        in_maps.append({"xt": xt, "xa": xa, "lab": shard})

    nc = _get_nc()
    res = run_bass_kernel_spmd(
        nc, in_maps, core_ids=list(range(NCORES)), trace=TRACE
    )
    LAST_RESULT = res

    out = np.empty((B, N_TOTAL, E), np.float32)
    for r in range(NCORES):
        lo = r * NS
        hi = min(lo + NS, N_TOTAL)
        out[:, lo:hi, :] = res.results[r]["m"][:, : hi - lo, :]
    return out


# revision 15
# speedup vs baseline: 3.8544x; 1.0187x over previous
"""CodeWiseAttention kernel for Trainium2 (8 NeuronCores, label-dim sharded).

m[b,n,:] = softmax(label_feature[n] @ x[b].T) @ x[b]

Sharding: label rows N=8922 split across 8 cores (1116/core; core 7 pads 6
rows); x replicated. All transposes/padding done on host; on-chip per core,
one global stream of 160 l-chunks (8 batches x 20 chunks of 128 rows):
  mm1 (fp16):  S[l,n] = xT[e,l].T @ labT[e,n]   3 matmuls of n-width 372
  exp (ScalarE): e_sb = bf16(exp(S - 30))       one call per l-chunk (N=1116)
  mm2 (bf16):  m_ps[n,e'] += e_sb[l,n-chunk].T @ xa[l,e']  9 matmuls, lagging
      the exp stream by 2 chunks; xa has a ones column so col 100 of U = Z.
  out: m = U/Z on VectorE straight from [n,e] layout, 2 DMA triggers/batch.

ScalarE's exp stream (1 elem/cycle/lane, cost = free-dim columns) is the
roofline: ~152us of pure streaming + per-call overhead. Everything else is
shaped to keep ScalarE gapless: mm1 of chunk g is emitted BEFORE mm2 of
chunk g-2 so the PE (in-order queue) produces scores the moment the s-buffer
frees; weight loads hide under matmuls via the PE's reorder window; input
DMAs trigger from the idle GpSimd queue and outputs from Sync (each trigger
costs ~0.6us of sequencer time); VectorE does the U/Z division.

fp16 for mm1 keeps score error ~4e-3 abs (exp amplifies score errors); bf16
for mm2 because exp values reach e^31 (fp16 would overflow). Measured rel
err vs f64 reference: ~4e-3 (threshold 2e-2).

The 9 m-accumulator slots (102 f32 cols) pack 5-per-bank into 2 PSUM banks.
Only the first slot of each bank uses start=True (clearing the whole bank's
has_written bits); the other slots' first matmuls then overwrite-where-unset
and all later chunks accumulate — 9 interleaved accumulation groups share 2
banks. The per-batch U snapshot to SBUF doubles as the WAR anchor that keeps
the next batch's matmuls out of m_ps while VectorE reads it (PE-write +
DVE-read of one PSUM bank is fatal; reading uninitialized PSUM is fatal too,
hence the one-time memset). PSUM: 2 x 3 banks scores + 2 banks U = 8.
"""
import numpy as np
from contextlib import ExitStack

import concourse.tile as tile
from concourse import bacc, mybir
from concourse.bass_utils import run_bass_kernel_spmd

F32 = mybir.dt.float32
F16 = mybir.dt.float16
BF16 = mybir.dt.bfloat16

B, L, E = 8, 2500, 100
LP = 2560          # L padded to 20 chunks of 128 (zero rows contribute nothing)
NCHUNK = LP // 128
NG = B * NCHUNK
N_TOTAL = 8922
NCORES = 8
NS = 1116          # label rows per core (core 7: 1110 real, 6 pad)
NCH = 372          # mm1 moving width (3 x 372 = 1116)
EA = E + 2         # x cols + ones col (100) + pad col (101)
PSB = 512          # psum bank stride in f32 elements
EXP_BIAS = -30.0
SLOT_P = [128] * 8 + [92]   # n-rows per m-accumulator slot (9 x 128 > 1116)

TRACE = False
LAST_RESULT = None

_NC = []


def _build():
    nc = bacc.Bacc("TRN2", target_bir_lowering=False, debug=False)
    xt_d = nc.dram_tensor("xt", [B, E, LP], F16, kind="ExternalInput").ap()
    xa_d = nc.dram_tensor("xa", [B, LP, EA], BF16, kind="ExternalInput").ap()
    lab_d = nc.dram_tensor("lab", [E, NS], F16, kind="ExternalInput").ap()
    m_d = nc.dram_tensor("m", [B, NS, E], F32, kind="ExternalOutput").ap()

    with tile.TileContext(nc) as tc, ExitStack() as ctx:
        consts = ctx.enter_context(tc.tile_pool(name="consts", bufs=1))
        xt_pool = ctx.enter_context(tc.tile_pool(name="xtp", bufs=2))
        xa_pool = ctx.enter_context(tc.tile_pool(name="xap", bufs=2))
        e_pool = ctx.enter_context(tc.tile_pool(name="ep", bufs=4))
        u_pool = ctx.enter_context(tc.tile_pool(name="up", bufs=2))
        o_pool = ctx.enter_context(tc.tile_pool(name="op", bufs=2))
        r_pool = ctx.enter_context(tc.tile_pool(name="rp", bufs=2))
        s_psum = ctx.enter_context(tc.tile_pool(name="sps", bufs=2, space="PSUM"))
        m_psum = ctx.enter_context(tc.tile_pool(name="mps", bufs=1, space="PSUM"))

        bias_sb = consts.tile([128, 1], F32)
        nc.vector.memset(bias_sb[:], EXP_BIAS)
        labT_sb = consts.tile([E, NS], F16)

        m_ps = m_psum.tile([128, 2, PSB], F32, name="m_ps")
        # one-time init so whole-tile reads below never touch uninitialized
        # PSUM; 1.0 (not 0) so the dummy-slot 1/Z stays finite
        nc.vector.memset(m_ps[:], 1.0)

        xt_tiles, xa_tiles, e_tiles = {}, {}, {}

        def dma_in(b):
            xt_tiles[b] = xt_pool.tile([E, LP], F16, tag="xt", name=f"xt{b}")
            xa_tiles[b] = xa_pool.tile(
                [128, NCHUNK, EA], BF16, tag="xa", name=f"xa{b}")
            nc.gpsimd.dma_start(out=xt_tiles[b][:], in_=xt_d[b])
            nc.gpsimd.dma_start(
                out=xa_tiles[b][:],
                in_=xa_d[b].rearrange("(c p) e -> p c e", p=128))

        def mm2_chunk(b, c, e_sb):
            xa_sb = xa_tiles[b]
            e_flat = e_sb[:].rearrange("p a b -> p (a b)")
            for jn in range(9):
                q, r2 = divmod(jn, 5)
                np_ = SLOT_P[jn]
                nc.tensor.matmul(
                    m_ps[0:np_, q, r2 * EA:(r2 + 1) * EA],
                    e_flat[:, jn * 128:jn * 128 + np_],
                    xa_sb[:, c, :],
                    start=(c == 0 and r2 == 0), stop=(c == NCHUNK - 1),
                    skip_group_check=True,
                )

        def out_path(b, src):
            # one strided reciprocal over all 10 Z columns (slot (1,4) is a
            # dummy), one broadcast multiply for U/Z, two DMA triggers
            rz = r_pool.tile([128, 2, 5], F32, tag="r")
            nc.vector.reciprocal(rz[:], src[:, :, E:EA * 5:EA])
            u4 = src[:, :, 0:EA * 5].rearrange(
                "p q (r c) -> p q r c", r=5)[:, :, :, 0:E]
            o_flat = o_pool.tile([128, 10 * E], F32, tag="o")
            o4 = o_flat[:].rearrange("p (q r c) -> p q r c", q=2, r=5)
            nc.vector.tensor_tensor(
                o4, u4, rz[:].to_broadcast([128, 2, 5, E]),
                mybir.AluOpType.mult)
            o3 = o_flat[:].rearrange("p (s c) -> p s c", s=10)
            nc.sync.dma_start(
                out=m_d[b, 0:1024, :].rearrange("(s p) e -> p s e", p=128),
                in_=o3[:, 0:8, :])
            nc.sync.dma_start(
                out=m_d[b, 1024:NS, :], in_=o3[0:NS - 1024, 8, :])

        def finish_batch(b):
            if b + 1 < B:
                # U snapshot: the WAR anchor keeping batch b+1's matmuls out
                # of m_ps until VectorE has read batch b's result
                u_sb = u_pool.tile([128, 2, PSB], F32, tag="u")
                nc.vector.tensor_copy(u_sb[:], m_ps[:])
                out_path(b, u_sb)
            else:
                out_path(b, m_ps)  # last batch: no later matmuls
            del xa_tiles[b]

        # prologue: chunk 0's deps (xt piece 1, labT) trigger first on the
        # Sync queue (its trigger is ~300ns cheaper than GpSimd's); the rest
        # streams from GpSimd in lands-in-time pieces
        xt_tiles[0] = xt_pool.tile([E, LP], F16, tag="xt", name="xt0")
        xa_tiles[0] = xa_pool.tile(
            [128, NCHUNK, EA], BF16, tag="xa", name="xa0")
        xa0_r = xa_d[0].rearrange("(c p) e -> p c e", p=128)
        nc.sync.dma_start(out=xt_tiles[0][:, 0:256], in_=xt_d[0][:, 0:256])
        nc.sync.dma_start(out=labT_sb[:], in_=lab_d)
        nc.gpsimd.dma_start(out=xa_tiles[0][:, 0:2, :], in_=xa0_r[:, 0:2, :])
        nc.gpsimd.dma_start(
            out=xt_tiles[0][:, 256:1024], in_=xt_d[0][:, 256:1024])
        nc.gpsimd.dma_start(out=xa_tiles[0][:, 2:8, :], in_=xa0_r[:, 2:8, :])
        nc.gpsimd.dma_start(
            out=xt_tiles[0][:, 1024:LP], in_=xt_d[0][:, 1024:LP])
        nc.gpsimd.dma_start(
            out=xa_tiles[0][:, 8:NCHUNK, :], in_=xa0_r[:, 8:NCHUNK, :])
        for g in range(NG + 2):
            b, c = divmod(g, NCHUNK)
            if g < NG:
                if c == 10 and b + 1 < B:
                    dma_in(b + 1)
                xt_sb = xt_tiles[b]
                s_ps = s_psum.tile([128, 3, PSB], F32, tag="s")
                for j3 in range(3):
                    nc.tensor.matmul(
                        s_ps[:, j3, 0:NCH],
                        xt_sb[:, c * 128:(c + 1) * 128],
                        labT_sb[:, j3 * NCH:(j3 + 1) * NCH],
                    )
                e_tiles[g] = e_pool.tile(
                    [128, 3, NCH], BF16, tag="e", name=f"e{g}")
                nc.scalar.activation(
                    e_tiles[g][:], s_ps[:, 0:3, 0:NCH],
                    mybir.ActivationFunctionType.Exp,
                    bias=bias_sb[:], scale=1.0,
                )
                if c == NCHUNK - 1:
                    del xt_tiles[b]
            if g >= 2:
                b2, c2 = divmod(g - 2, NCHUNK)
                mm2_chunk(b2, c2, e_tiles.pop(g - 2))
                if c2 == NCHUNK - 1:
                    finish_batch(b2)
    nc.compile()
    return nc


def _get_nc():
    if not _NC:
        _NC.append(_build())
    return _NC[0]


def kernel(x, label_feature):
    global LAST_RESULT
    np_f16 = mybir.dt.np(F16)
    np_bf16 = mybir.dt.np(BF16)
    x = np.ascontiguousarray(np.asarray(x, dtype=np.float32))
    lf = np.ascontiguousarray(np.asarray(label_feature, dtype=np.float32))
    assert x.shape == (B, L, E) and lf.shape == (N_TOTAL, E)

    xt = np.zeros((B, E, LP), np_f16)
    xt[:, :, :L] = x.transpose(0, 2, 1).astype(np_f16)
    xa = np.zeros((B, LP, EA), np_bf16)
    xa[:, :L, :E] = x.astype(np_bf16)
    xa[:, :L, E] = 1.0

    in_maps = []
    for r in range(NCORES):
        lo = r * NS
        hi = min(lo + NS, N_TOTAL)
        shard = np.zeros((E, NS), np_f16)
        shard[:, : hi - lo] = lf[lo:hi].T.astype(np_f16)
        in_maps.append({"xt": xt, "xa": xa, "lab": shard})

    nc = _get_nc()
    res = run_bass_kernel_spmd(
        nc, in_maps, core_ids=list(range(NCORES)), trace=TRACE
    )
    LAST_RESULT = res

    out = np.empty((B, N_TOTAL, E), np.float32)
    for r in range(NCORES):
        lo = r * NS
        hi = min(lo + NS, N_TOTAL)
        out[:, lo:hi, :] = res.results[r]["m"][:, : hi - lo, :]
    return out
